# revision 20
# baseline (speedup 1.0000x reference)
"""2-layer GCN (GCNConv -> ReLU -> GCNConv) on 8 Trainium2 NeuronCores.

conv = dinv * (sum_{src->dst} y'[src] + y'[dst]) + b with y' = dinv*(x@W).
Propagation = unweighted gather-sum of pre-scaled rows.

v2 gather core: instead of one indirect DMA per 128 rows (~1.5us each on the
Pool engine), use InstDMAGatherAnt (the MoE token-dispatch gather): one
instruction gathers 1024 rows (ring limit), round-robined over 4 SWDGE queues.
The int16-index limit (32768 rows) is met by storing the table as QUADS: one
256B-stride row holds 4 nodes' 16 bf16 features (idx = node>>2), and a
host-provided {0,1} mask selects the right sub-row on DVE before the k-sum.
Geometry: chunks of 1024 nodes (p=128 x c=8) so each 1024-slot call is exactly
one k-row of the chunk; per-call DVE: masked-mult -> reduce over quad -> acc.

L2/L3 are written in raw Block style (not Tile): Tile does not attach the DMA
completion semaphore InstDMAGatherAnt needs for ring reclaim, which corrupts
back-to-back gathers.  Engines sync via explicit cumulative semaphores.
"""

import os
import sys

for _p in ("/opt/trn_rl_repo", "/root/.axon_site/_ro/trn_rl_repo"):
    if os.path.isdir(_p) and _p not in sys.path:
        sys.path.append(_p)

from contextlib import ExitStack

import numpy as np
import ml_dtypes

import concourse.bass as bass
import concourse.bacc as bacc
import concourse.tile as tile
from concourse import mybir
from concourse import ap_utils
from concourse.bass import MemorySpace
from concourse.bass_utils import run_bass_kernel_spmd

dt = mybir.dt
F32 = dt.float32
BF16 = dt.bfloat16
I16 = dt.int16
ALU = mybir.AluOpType
AXL = mybir.AxisListType

N = 100000          # real nodes
F = 256             # input features
H = 16              # hidden
O = 40              # classes
NCORES = 8
P = 128
C = 8               # node columns per partition per chunk
NODES_PER_CHUNK = P * C          # 1024
CHUNKS = 13
PC = NODES_PER_CHUNK * CHUNKS    # 13312 nodes per core
NPAD = PC * NCORES               # 106496 padded node space
QROWS = NPAD // 4                # quad rows in gather tables
ZR = N                           # rows >= N are all-zero
NIDX = 1024                      # gather slots per call (ring limit)
NQ = 4                           # SWDGE queues

_TRACE = bool(os.environ.get("GNN_TRACE"))
_EXEC_NS = []


def dma_gather_raw(g, out_ap, in_ap, idxs_ap, num_idxs, elem_size, elem_step,
                   queue_num):
    """bass dma_gather minus the elem_size%256 assert (non-transpose ucode
    supports any elem size; only the row stride must be a 256B multiple)."""
    assert idxs_ap.dtype == mybir.dt.int16
    assert in_ap.dtype == out_ap.dtype
    assert in_ap.space == MemorySpace.DRAM
    assert ap_utils.ap_is_contiguous(in_ap.ap[1:])
    assert ap_utils.ap_is_contiguous(out_ap.ap[1:])
    assert ap_utils.ap_is_contiguous(idxs_ap.ap[1:])
    assert in_ap.ap[-1][1] == out_ap.ap[-1][1] == elem_size
    assert in_ap.ap[0][0] == elem_step
    stride_bytes = elem_step * mybir.dt.size(in_ap.dtype)
    stride_bytes_256 = stride_bytes // 256
    assert stride_bytes_256 * 256 == stride_bytes and stride_bytes_256 < 256
    _in_ap = g.lower_ap_dma(in_ap, for_custom_bir_dma=True)
    _idxs_ap = g.lower_ap(idxs_ap)
    _out_ap = g.lower_ap(out_ap)
    return g.add_instruction(
        mybir.InstDMAGatherAnt(
            name=g.bass.get_next_instruction_name(),
            ins=[*_in_ap, _idxs_ap, g.lower_val_access(g.to_reg(num_idxs))],
            outs=[_out_ap],
            transpose=False,
            num_idxs=num_idxs,
            elem_size=elem_size,
            stride_bytes_256=stride_bytes_256,
            gen_mode=0,
            single_packet=True,
            queue_num=queue_num,
            sbuf_tokens_per_rank=0,
            sbuf_free_dim_per_rank=0,
            sbuf_free_dim_pad_per_rank=0,
            sbuf_byte_offset=0,
        )
    )


# --------------------------------------------------------------------------
# device programs
# --------------------------------------------------------------------------

def build_l1():
    """y' = dinv * (x @ W1) for this core's PC contiguous rows -> bf16."""
    nc = bacc.Bacc()
    xT = nc.declare_dram_parameter("xT", [F, PC], F32, isOutput=False)
    w1 = nc.declare_dram_parameter("w1", [F, H], F32, isOutput=False)
    dinv = nc.declare_dram_parameter("dinv", [PC], F32, isOutput=False)
    yp = nc.declare_dram_parameter("yp", [PC, H], BF16, isOutput=True)

    with tile.TileContext(nc) as tc:
        with (
            tc.tile_pool(name="w", bufs=1) as wp,
            tc.tile_pool(name="x", bufs=3) as xp,
            tc.tile_pool(name="d", bufs=2) as dp,
            tc.tile_pool(name="y", bufs=3) as yo,
            tc.tile_pool(name="ps", bufs=4, space="PSUM") as pp,
        ):
            w1a = wp.tile([P, H], F32, tag="w1a")
            w1b = wp.tile([P, H], F32, tag="w1b")
            nc.sync.dma_start(out=w1a[:], in_=w1[0:P, :])
            nc.sync.dma_start(out=w1b[:], in_=w1[P:F, :])

            for s in range(CHUNKS):
                cols = slice(s * NODES_PER_CHUNK, (s + 1) * NODES_PER_CHUNK)
                xa = xp.tile([P, NODES_PER_CHUNK], F32, tag="xa")
                xb = xp.tile([P, NODES_PER_CHUNK], F32, tag="xb")
                nc.sync.dma_start(out=xa[:], in_=xT[0:P, cols])
                nc.sync.dma_start(out=xb[:], in_=xT[P:F, cols])
                dv = dp.tile([P, C], F32, tag="dv")
                nc.sync.dma_start(
                    out=dv[:],
                    in_=dinv[cols].rearrange("(t p) -> p t", p=P),
                )
                yt = yo.tile([P, C * H], BF16, tag="yt")
                for t in range(C):
                    ps = pp.tile([P, H], F32, tag="ps")
                    nc.tensor.matmul(
                        out=ps[:], lhsT=xa[:, t * P:(t + 1) * P], rhs=w1a[:],
                        start=True, stop=False,
                    )
                    nc.tensor.matmul(
                        out=ps[:], lhsT=xb[:, t * P:(t + 1) * P], rhs=w1b[:],
                        start=False, stop=True,
                    )
                    nc.vector.tensor_scalar(
                        out=yt[:, t * H:(t + 1) * H], in0=ps[:],
                        scalar1=dv[:, t:t + 1], scalar2=None, op0=ALU.mult,
                    )
                nc.sync.dma_start(
                    out=yp[cols, :].rearrange("(t p) h -> p t h", p=P),
                    in_=yt[:].rearrange("p (t h) -> p t h", h=H),
                )
    nc.compile()
    return nc


def build_gather_layer(ks, final):
    """Gather-sum layer over the permuted node layout.

    final=False (L2): hp = bf16[ dinv * relu(dinv*agg + b1) * dinv ]... i.e.
        h' = dinv * relu(dinv*agg + b1), stored bf16.
    final=True  (L3): out = f32[ (dinv*agg) @ W2 + b2 ].
    """
    TOT = int(np.sum(ks))
    cb = np.concatenate([[0], np.cumsum(ks)]).astype(int)  # calls_before
    KMAX = int(np.max(ks))
    B = 48  # gather buffer ring

    nc = bacc.Bacc(num_swdge_queues=NQ)
    table = nc.declare_dram_parameter("table", [QROWS, P], BF16,
                                      isOutput=False)
    idxs = nc.declare_dram_parameter("idxs", [P, TOT * 64], I16, isOutput=False)
    masks = nc.declare_dram_parameter("masks", [P, TOT * 32], BF16,
                                      isOutput=False)
    dinvp = nc.declare_dram_parameter("dinvp", [P, CHUNKS * C], F32,
                                      isOutput=False)
    if final:
        w2r = nc.declare_dram_parameter("w2r", [P, H * C * O], F32,
                                        isOutput=False)
        b2r = nc.declare_dram_parameter("b2r", [P, C * O], F32, isOutput=False)
        out = nc.declare_dram_parameter("out", [PC, O], F32, isOutput=True)
    else:
        b1r = nc.declare_dram_parameter("b1r", [P, C * H], F32,
                                        isOutput=False)
        out = nc.declare_dram_parameter("out", [PC, H], BF16, isOutput=True)
    OW = O if final else H

    with ExitStack() as st:
        blk = st.enter_context(nc.Block())
        gb = [st.enter_context(nc.sbuf_tensor(f"gb{i}", [P, C * 64], BF16))
              for i in range(B)]
        idxb = [st.enter_context(nc.sbuf_tensor(f"idxb{i}", [P, KMAX * 64], I16))
                for i in range(2)]
        maskb = [st.enter_context(nc.sbuf_tensor(f"maskb{i}", [P, KMAX * 32],
                                                 BF16)) for i in range(2)]
        dvb = [st.enter_context(nc.sbuf_tensor(f"dvb{i}", [P, C], F32))
               for i in range(2)]
        tm = st.enter_context(nc.sbuf_tensor("tm", [P, C * 64], BF16))
        zer = st.enter_context(nc.sbuf_tensor("zer", [P, C * H], F32))
        red = st.enter_context(nc.sbuf_tensor("red", [P, C * 2 * H], F32))
        acc = st.enter_context(nc.sbuf_tensor("acc", [P, C * H], F32))
        ob = [st.enter_context(nc.sbuf_tensor(f"ob{i}", [P, C * OW],
                                              F32 if final else BF16))
              for i in range(2)]
        cst = st.enter_context(nc.sbuf_tensor("cst", [P, H * C * O + C * O],
                                              F32))
        if final:
            tmpo = st.enter_context(nc.sbuf_tensor("tmpo", [P, C * O], F32))
        isem = st.enter_context(nc.semaphore("isem"))
        RS = 13  # rotating sems per queue; NQ*RS > B so no concurrent reuse
        qsems = [[st.enter_context(nc.semaphore(f"qsem{q}_{s}"))
                  for s in range(RS)] for q in range(NQ)]
        csem = st.enter_context(nc.semaphore("csem"))
        hsem = st.enter_context(nc.semaphore("hsem"))
        osem = st.enter_context(nc.semaphore("osem"))

        @blk.sync
        def _(sp: bass.BassEngine):
            if final:
                sp.dma_start(cst[:, 0:H * C * O], w2r[:, :]).then_inc(isem, 16)
                sp.dma_start(cst[:, H * C * O:], b2r[:, :]).then_inc(isem, 16)
            else:
                sp.dma_start(cst[:, 0:C * H], b1r[:, :]).then_inc(isem, 16)
            nload = 2 if final else 1
            for ch in range(CHUNKS):
                K = ks[ch]
                if ch >= 2:
                    # idx buf (ch-2)%2 reused: DVE consumed chunk ch-2's calls
                    # implies their gathers (and Pool-serial gens) are done
                    sp.wait_ge(csem, int(cb[ch - 1]))
                    # mask/dv bufs reused: chunk ch-2 fully consumed by DVE
                    sp.wait_ge(hsem, ch - 1)
                sp.dma_start(
                    idxb[ch % 2][:, 0:K * 64],
                    idxs[:, cb[ch] * 64:cb[ch + 1] * 64],
                ).then_inc(isem, 16)
                sp.dma_start(
                    maskb[ch % 2][:, 0:K * 32],
                    masks[:, cb[ch] * 32:cb[ch + 1] * 32],
                ).then_inc(isem, 16)
                sp.dma_start(
                    dvb[ch % 2][:],
                    dinvp[:, ch * C:(ch + 1) * C],
                ).then_inc(isem, 16)
                if ch >= 1:
                    sp.wait_ge(hsem, ch)
                    prows = slice((ch - 1) * NODES_PER_CHUNK,
                                  ch * NODES_PER_CHUNK)
                    sp.dma_start(
                        out[prows, :].rearrange("(c p) o -> p c o", p=P),
                        ob[(ch - 1) % 2][:].rearrange("p (c o) -> p c o",
                                                      o=OW),
                    ).then_inc(osem, 16)
            sp.wait_ge(hsem, CHUNKS)
            prows = slice((CHUNKS - 1) * NODES_PER_CHUNK, PC)
            sp.dma_start(
                out[prows, :].rearrange("(c p) o -> p c o", p=P),
                ob[(CHUNKS - 1) % 2][:].rearrange("p (c o) -> p c o", o=OW),
            ).then_inc(osem, 16)
            sp.wait_ge(osem, 16 * CHUNKS)
            sp.wait_ge(csem, TOT)

        @blk.gpsimd
        def _(gp: bass.BassGpSimd):
            nload = 3 + (2 if final else 1)
            for ch in range(CHUNKS):
                K = ks[ch]
                gp.wait_ge(isem, 16 * (nload + 3 * ch))
                for k in range(K):
                    j = int(cb[ch]) + k
                    if j >= B:
                        gp.wait_ge(csem, j - B + 1)
                    dma_gather_raw(
                        gp,
                        gb[j % B][:].rearrange("p (c e) -> p c e", e=64),
                        table[:, 0:64],
                        idxb[ch % 2][:, k * 64:(k + 1) * 64],
                        NIDX, 64, P,
                        queue_num=j % NQ,
                    ).then_inc(qsems[j % NQ][(j // NQ) % RS], 16)

        @blk.vector
        def _(v: bass.BassVectorEngine):
            nload = 3 + (2 if final else 1)
            v.memset(zer[:], 0.0)
            for ch in range(CHUNKS):
                K = ks[ch]
                v.wait_ge(isem, 16 * (nload + 3 * ch))
                v.memset(acc[:], 0.0)
                for k in range(K):
                    j = int(cb[ch]) + k
                    v.wait_ge(qsems[j % NQ][(j // NQ) % RS],
                              16 * (j // (NQ * RS) + 1))
                    # tm = g * mask  (mask selects 1 of 4 sub-rows)
                    g3 = gb[j % B][:].rearrange("p (s h) -> p s h", h=H)
                    m3 = (maskb[ch % 2][:, k * 32:(k + 1) * 32]
                          .rearrange("p (s one) -> p s one", one=1)
                          .to_broadcast([P, 4 * C, H]))
                    v.tensor_tensor(
                        out=tm[:].rearrange("p (s h) -> p s h", h=H),
                        in0=g3, in1=m3, op=ALU.mult,
                    ).then_inc(csem, 1)
                    # red[p, c, h] = sum_q tm[p, c, q, h]  (reduce over the
                    # strided q axis; exact in bf16: <=1 nonzero per quad)
                    v.tensor_reduce(
                        out=red[:, 0:C * H].rearrange("p (c h) -> p c h", h=H),
                        in_=tm[:].rearrange("p (c q h) -> p c h q", q=4, h=H),
                        axis=AXL.X, op=ALU.add,
                    )
                    v.tensor_tensor(out=acc[:], in0=acc[:],
                                    in1=red[:, 0:C * H], op=ALU.add)
                # post: scale by dinv etc.
                acc3 = acc[:].rearrange("p (c h) -> p c h", h=H)
                dv3 = dvb[ch % 2][:].unsqueeze(2).to_broadcast([P, C, H])
                v.tensor_tensor(out=acc3, in0=acc3, in1=dv3, op=ALU.mult)
                if ch >= 2:
                    v.wait_ge(osem, 16 * (ch - 1))
                o3 = ob[ch % 2][:].rearrange("p (c o) -> p c o", o=OW)
                if final:
                    # out = acc @ W2 + b2 via 16 broadcast mult-adds on DVE
                    t3 = tmpo[:].rearrange("p (c o) -> p c o", o=O)
                    b23 = (cst[:, H * C * O:]
                           .rearrange("p (c o) -> p c o", o=O))
                    for h in range(H):
                        w2h = (cst[:, h * C * O:(h + 1) * C * O]
                               .rearrange("p (c o) -> p c o", o=O))
                        gsh = (acc3[:, :, h:h + 1].to_broadcast([P, C, O]))
                        if h == 0:
                            v.tensor_tensor(out=o3, in0=gsh, in1=w2h,
                                            op=ALU.mult)
                        else:
                            v.tensor_tensor(out=t3, in0=gsh, in1=w2h,
                                            op=ALU.mult)
                            v.tensor_tensor(out=o3, in0=o3, in1=t3,
                                            op=ALU.add)
                    v.tensor_tensor(out=o3, in0=o3, in1=b23,
                                    op=ALU.add).then_inc(hsem, 1)
                elif os.environ.get("GNN_DUMP_ACC"):
                    v.tensor_tensor(out=o3, in0=acc3, in1=acc3,
                                    op=ALU.bypass).then_inc(hsem, 1)
                else:
                    b13 = cst[:, 0:C * H].rearrange("p (c h) -> p c h", h=H)
                    v.tensor_tensor(out=acc3, in0=acc3, in1=b13, op=ALU.add)
                    if not os.environ.get("GNN_NO_RELU"):
                        v.tensor_tensor(out=acc[:], in0=acc[:], in1=zer[:],
                                        op=ALU.max)
                    v.tensor_tensor(out=o3, in0=acc3, in1=dv3,
                                    op=ALU.mult).then_inc(hsem, 1)
    nc.compile()
    return nc


# --------------------------------------------------------------------------
# host orchestration
# --------------------------------------------------------------------------

def _install_trace_shim():
    import types
    import contextlib
    import ctypes

    if "antenv.axon_hooks" not in sys.modules:
        lib = ctypes.CDLL("/opt/axon/libaxon_pjrt.so")
        lib.axon_start_nrt_profile.argtypes = [
            ctypes.POINTER(ctypes.c_int64), ctypes.c_size_t]
        lib.axon_start_nrt_profile.restype = ctypes.c_int64
        lib.axon_stop_nrt_profile.argtypes = [ctypes.c_char_p]
        lib.axon_stop_nrt_profile.restype = ctypes.c_int64

        @contextlib.contextmanager
        def _hook(output_dir, device_ids):
            import jax
            jax.devices()
            if device_ids:
                ids = (ctypes.c_int64 * len(device_ids))(*device_ids)
                rc = lib.axon_start_nrt_profile(ids, len(device_ids))
            else:
                rc = lib.axon_start_nrt_profile(None, 0)
            if rc != 0:
                raise RuntimeError(f"axon_start_nrt_profile rc={rc}")
            try:
                yield
            finally:
                n = lib.axon_stop_nrt_profile(str(output_dir).encode())
                print(f"profile: {n} file(s) -> {output_dir}", file=sys.stderr)

        mod = types.ModuleType("antenv.axon_hooks")
        mod.get_axon_ntff_profile_hook = lambda: _hook
        mod.set_axon_ntff_profile_hook = lambda h: None
        sys.modules["antenv.axon_hooks"] = mod

    import concourse.bass_utils as bu
    bu.upload_artifacts = lambda tmpdir: "local://skipped"


def _run(nc, in_maps, label):
    if _TRACE:
        _install_trace_shim()
        res = run_bass_kernel_spmd(
            nc, in_maps, list(range(NCORES)), trace=True, trace_cores=[0],
        )
        print(f"[{label}] exec_time_ns={res.exec_time_ns}", file=sys.stderr)
        _EXEC_NS.append((label, res.exec_time_ns))
        if res.instructions_and_trace is not None:
            print(f"[{label}] trace={res.instructions_and_trace[1]}",
                  file=sys.stderr)
        return res.results
    return run_bass_kernel_spmd(nc, in_maps, list(range(NCORES))).results


def _quad_table(rows16):
    """[NPAD, 16] bf16 -> padded quad table [QROWS, 128] bf16 (cols 64.. zero)."""
    t = np.zeros((QROWS, P), dtype=ml_dtypes.bfloat16)
    t[:, 0:64] = np.asarray(rows16).reshape(QROWS, 64)
    return t


def prep(edge_index):
    """All graph-dependent host prep; returns dict of per-core arrays."""
    src = np.ascontiguousarray(edge_index[0]).astype(np.int64)
    dst = np.ascontiguousarray(edge_index[1]).astype(np.int64)
    E = src.shape[0]

    counts = np.bincount(dst, minlength=NPAD).astype(np.int64)
    dinv = np.zeros(NPAD, np.float32)
    dinv[:N] = 1.0 / np.sqrt((counts[:N] + 1).astype(np.float64))

    order_e = np.argsort(dst, kind="stable")
    src_sorted = src[order_e].astype(np.int64)
    starts = np.zeros(NPAD + 1, np.int64)
    np.cumsum(counts, out=starts[1:])

    ordern = np.argsort(-counts, kind="stable").astype(np.int64)
    blocks = ordern.reshape(CHUNKS, NODES_PER_CHUNK * NCORES)
    node_layout = blocks.reshape(
        CHUNKS, NODES_PER_CHUNK, NCORES).transpose(2, 0, 1)  # [core, ch, i]
    ks = [int(counts[blocks[ch]].max()) + 1 for ch in range(CHUNKS)]
    TOT = int(np.sum(ks))

    idx_cores, mask_cores, dinvp_cores = [], [], []
    for core in range(NCORES):
        idx_arr = np.empty((P, TOT * 64), np.int16)
        mask_arr = np.zeros((P, TOT * 32), ml_dtypes.bfloat16)
        call = 0
        for ch in range(CHUNKS):
            nodes = node_layout[core, ch]            # [1024] i order (c*128+p)
            K = ks[ch]
            kk = np.arange(K, dtype=np.int64)
            pos = starts[nodes][:, None] + kk[None, :] - 1
            valid = (kk[None, :] >= 1) & (kk[None, :] <= counts[nodes][:, None])
            vals = np.where(
                kk[None, :] == 0,
                nodes[:, None],
                np.where(valid, src_sorted[np.clip(pos, 0, E - 1)], ZR),
            )                                        # [1024, K]
            use = (kk[None, :] == 0) | valid         # mask on
            for k in range(K):
                v = vals[:, k]
                idx16 = (v >> 2).astype(np.int16)    # quad idx
                wrapped = np.tile(idx16.reshape(64, 16).T, (8, 1))  # [128,64]
                idx_arr[:, call * 64:(call + 1) * 64] = wrapped
                m = np.zeros((NODES_PER_CHUNK, 4), np.float32)
                m[np.arange(NODES_PER_CHUNK), v & 3] = use[:, k]
                # i = c*128 + p  ->  [p, c, 4]
                m = m.reshape(C, P, 4).transpose(1, 0, 2).reshape(P, 32)
                mask_arr[:, call * 32:(call + 1) * 32] = m
                call += 1
        assert call == TOT
        idx_cores.append(idx_arr)
        mask_cores.append(np.ascontiguousarray(mask_arr))
        dvi = dinv[node_layout[core].reshape(-1)]          # [PC] i order
        dinvp_cores.append(np.ascontiguousarray(
            dvi.reshape(CHUNKS, C, P).transpose(2, 0, 1).reshape(P, -1)))

    layout_flat = [node_layout[core].reshape(-1) for core in range(NCORES)]
    return dict(
        dinv=dinv, ks=ks, idx=idx_cores, mask=mask_cores,
        dinvp=dinvp_cores, layout_flat=layout_flat,
    )


def kernel(x, edge_index, W1, b1, W2, b2):
    x = np.ascontiguousarray(np.asarray(x, dtype=np.float32))
    W1 = np.ascontiguousarray(np.asarray(W1, dtype=np.float32))
    b1 = np.asarray(b1, dtype=np.float32).reshape(-1)
    W2 = np.ascontiguousarray(np.asarray(W2, dtype=np.float32))
    b2 = np.asarray(b2, dtype=np.float32).reshape(-1)

    pp = prep(np.asarray(edge_index))
    dinv, ks = pp["dinv"], pp["ks"]

    xT = np.zeros((F, NPAD), np.float32)
    xT[:, :N] = x.T
    b1r = np.ascontiguousarray(np.tile(b1[None, :], (P, C)))
    b2r = np.ascontiguousarray(np.tile(b2[None, :], (P, C)))
    # w2r[p, h*C*O + c*O + o] = W2[h, o]
    w2r = np.ascontiguousarray(
        np.tile(W2[:, None, :], (1, C, 1)).reshape(1, -1).repeat(P, axis=0))

    # ---- L1 ----
    nc1 = build_l1()
    maps1 = [
        {
            "xT": np.ascontiguousarray(xT[:, core * PC:(core + 1) * PC]),
            "w1": W1,
            "dinv": np.ascontiguousarray(dinv[core * PC:(core + 1) * PC]),
        }
        for core in range(NCORES)
    ]
    r1 = _run(nc1, maps1, "L1")
    ypad = np.concatenate(
        [np.asarray(r1[i]["yp"]) for i in range(NCORES)], axis=0)
    tbl1 = _quad_table(ypad)

    # ---- L2 ----
    nc2 = build_gather_layer(ks, final=False)
    maps2 = [
        {"table": tbl1, "idxs": pp["idx"][core], "masks": pp["mask"][core],
         "dinvp": pp["dinvp"][core], "b1r": b1r}
        for core in range(NCORES)
    ]
    r2 = _run(nc2, maps2, "L2")
    hpad = np.zeros((NPAD, H), ml_dtypes.bfloat16)
    for core in range(NCORES):
        hpad[pp["layout_flat"][core]] = np.asarray(r2[core]["out"])
    tbl2 = _quad_table(hpad)

    # ---- L3 ----
    nc3 = build_gather_layer(ks, final=True)
    maps3 = [
        {"table": tbl2, "idxs": pp["idx"][core], "masks": pp["mask"][core],
         "dinvp": pp["dinvp"][core], "w2r": w2r, "b2r": b2r}
        for core in range(NCORES)
    ]
    r3 = _run(nc3, maps3, "L3")
    outp = np.zeros((NPAD, O), np.float32)
    for core in range(NCORES):
        outp[pp["layout_flat"][core]] = np.asarray(r3[core]["out"])
    return np.ascontiguousarray(outp[:N])
